# revision 9
# baseline (speedup 1.0000x reference)
"""GCN layer (normalize -> BN(eval) -> Linear -> SpMM -> LeakyReLU) on 8 TRN2 cores.

Self-contained: host-side preprocessing (sharding / edge sorting / BN folding),
Bass/Tile program builder, and SPMD runner.

Strategy (per core, SPMD):
  - nodes sharded 8 x 6250 (padded to 6272 = 49*128 local rows)
  - phase 1: X_shard = rn * (H_shard @ W') + b'   (bf16, PE matmul; row-norm via
    ones-matmul + Rsqrt; BN folded into W'/b' on host)
  - X AllGathered in TWO slices (locals [0,3200) -> table A [25600,128],
    [3200,6272) -> table B [24576,128]); the slice split doubles as the int16
    gather-index split and lets AG1 overlap the tail of phase 1.
  - phase 3: edges sorted by (dest window of 128 rows, table half); chunks of
    128 edges; chunks per (win, half) = ceil(max-over-cores/128). Windows are
    grouped in quads; per (quad, half) the chunks form one contiguous idx
    segment gathered with <=1024-idx dma_gather calls round-robined over 4
    SWDGE queues (~3ns/desc vs ~9 single-queue; fewer calls cut the ~2.5us
    Pool-engine issue cost per call). Host-prebuilt scatter matrices
    S[128e,128d] (val at dloc, bf16) stream from DRAM per segment; one PE
    matmul per chunk accumulates into the window's full-bank [128,512] PSUM
    tile (bank per window => interleaved window lifetimes stay sound).
    Epilogue = one scalar-engine Lrelu activation PSUM->SBUF, then DMA out.
"""

import os
import sys

import numpy as np

for _p in ("/opt/trn_rl_repo", "/root/.axon_site/_ro/trn_rl_repo"):
    if _p not in sys.path and os.path.isdir(_p):
        sys.path.insert(0, _p)

import ml_dtypes  # noqa: E402

BF16 = ml_dtypes.bfloat16

# ---------------- problem constants (hardcoded per contract) ----------------
N = 50000
E = 800000
DIN = 128
DOUT = 64
BN_EPS = 1e-5
SLOPE = 0.01

NCORE = 8
RPC = N // NCORE          # 6250 real rows per core
LPAD = 6272               # 49*128 padded local rows
WIN = 128                 # dest-window rows (S matrix width / psum partitions)
NWIN = LPAD // WIN        # 49 windows/core
SPLITA = 3200             # local rows [0, 3200) -> table A (25 phase-1 chunks)
SPLITB = LPAD - SPLITA    # 3072 -> table B (24 chunks)
TA = NCORE * SPLITA       # 25600 rows (< 32768: int16-addressable)
TB = NCORE * SPLITB       # 24576 rows
XW = 128                  # X rows padded to 128 bf16 cols (256B stride)
NQ = 4                    # SWDGE queues (max 4)
CALLCAP = 8               # gather chunks per dma_gather call (1024 idx cap)
QUADW = 8                 # windows per gather-segment group (= 8 psum banks)


def _quads():
    qs = []
    w = 0
    while w < NWIN:
        qs.append(list(range(w, min(w + QUADW, NWIN))))
        w += QUADW
    return qs


def _schedule(nch):
    """nch [NWIN, 2] -> (colmap [NWIN,2], segments, nchunk, chunk_meta).

    Chunk columns ordered (quad, half, window, k). segments = per (quad, half):
    (col0, count, half). chunk_meta[col] = (win, start, stop)."""
    colmap = np.zeros((NWIN, 2), np.int64)
    segments = []
    col = 0
    for quad in _quads():
        for h in (0, 1):
            c0 = col
            for w in quad:
                colmap[w, h] = col
                col += int(nch[w, h])
            if col > c0:
                segments.append((c0, col - c0, h))
    nchunk = col
    meta = [None] * nchunk
    for w in range(NWIN):
        tot = int(nch[w, 0] + nch[w, 1])
        if tot == 0:
            continue
        ci = 0
        for h in (0, 1):
            for k in range(int(nch[w, h])):
                c = int(colmap[w, h]) + k
                meta[c] = (w, ci == 0, ci == tot - 1)
                ci += 1
    return colmap, segments, nchunk, meta


# ---------------- host preprocessing ----------------
def host_prep(H, rows, cols, vals, gamma, beta, run_mean, run_var, W, b):
    """Build the 8 per-core input maps + the static chunk schedule."""
    H = np.asarray(H, np.float32)
    rows = np.asarray(rows, np.int64)
    cols = np.asarray(cols, np.int64)
    vals = np.asarray(vals, np.float32)

    # BN fold: X = Hn @ W' + b'  with W' = diag(scale) W, b' = b + (beta-mean*scale)W
    scale = np.asarray(gamma, np.float32) / np.sqrt(np.asarray(run_var, np.float32) + BN_EPS)
    Wp = (np.asarray(W, np.float32) * scale[:, None]).astype(BF16)          # [128, 64]
    bp = (np.asarray(b, np.float32)
          + (np.asarray(beta, np.float32) - np.asarray(run_mean, np.float32) * scale)
          @ np.asarray(W, np.float32)).astype(np.float32)                    # [64]
    bp_tile = np.tile(bp[None, :], (128, 1)).astype(np.float32)              # [128, 64]

    core = rows // RPC
    lr = rows - core * RPC              # local dest row, 0..6249
    win = lr // WIN                     # 0..48
    dloc = lr - win * WIN               # 0..127
    m_src = cols // RPC
    lsrc = cols - m_src * RPC           # local source row, 0..6249
    half = (lsrc >= SPLITA).astype(np.int64)
    loc = np.where(half == 0, m_src * SPLITA + lsrc,
                   m_src * SPLITB + (lsrc - SPLITA))

    # ---- per-(core, win, half) slot assignment ----
    key = (core * NWIN + win) * 2 + half
    order = np.argsort(key, kind="stable")
    counts = np.bincount(key, minlength=NCORE * NWIN * 2)
    starts = np.zeros(NCORE * NWIN * 2 + 1, np.int64)
    np.cumsum(counts, out=starts[1:])
    j_in_run = np.arange(E, dtype=np.int64) - starts[key[order]]

    c3 = counts.reshape(NCORE, NWIN, 2)
    m8 = c3.max(axis=0)                           # [NWIN, 2]
    nch = ((m8 + 127) // 128).astype(np.int64)    # chunks per (win, half)
    colmap, segments, nchunk, _meta = _schedule(nch)

    e_core = core[order]
    e_win = win[order]
    e_half = half[order]
    e_dloc = dloc[order]
    e_loc = loc[order]
    e_val = vals[order]
    e_k = j_in_run // 128
    lane = (j_in_run % 128).astype(np.int64)
    col = colmap[e_win, e_half] + e_k

    idx16 = np.zeros((NCORE, 128, nchunk), np.int16)
    idx16[e_core, lane, col] = e_loc.astype(np.int16)
    # pre-built scatter matrices: svals[core, lane, col*128 + dloc] = val
    svals = np.zeros((NCORE, 128, nchunk * 128), BF16)
    svals[e_core, lane, col * 128 + e_dloc] = e_val.astype(BF16)

    # wrapped-16 idx, built per chunk (chunk c occupies cols [8c, 8c+8)):
    # idx_w[core, 16g + l%16, 8c + l//16] = idx16[core, l, c]
    tmp = idx16.reshape(NCORE, 8, 16, nchunk).transpose(0, 2, 3, 1)  # [NC,16,c,8]
    tmp = tmp.reshape(NCORE, 16, nchunk * 8)
    idx_w = np.ascontiguousarray(np.tile(tmp, (1, 8, 1)))

    # ---- H transposed shards, bf16 ----
    in_maps = []
    for m in range(NCORE):
        ht = np.zeros((DIN, LPAD), BF16)
        ht[:, :RPC] = H[m * RPC:(m + 1) * RPC].T.astype(BF16)
        ht[0, RPC:] = BF16(1.0)  # pad rows get norm 1 -> finite rn, X never read
        in_maps.append(dict(
            ht=ht,
            wp=Wp,
            bp=bp_tile,
            idx=np.ascontiguousarray(idx_w[m]),
            svals=np.ascontiguousarray(svals[m]),
        ))
    meta = dict(nch=tuple(int(x) for x in nch.reshape(-1)), nchunk=nchunk)
    return in_maps, meta


# ---------------- bass program ----------------
def _dma_gather_128(eng, out_ap, in_ap, idxs_ap, num_idxs, num_idxs_reg,
                    elem_size, elem_step, queue_num=0, prepare_only=False,
                    sem=None):
    """bass's dma_gather minus the 256B elem assert (transpose-only per the
    ucode; non-transpose packets are byte-granular, only the table row STRIDE
    must be a multiple of 256B). DRAM-source, non-transpose only."""
    import concourse.mybir as mybir
    from concourse.bass import round_up_to_multiple, exact_div
    from concourse import ap_utils

    assert idxs_ap.dtype == mybir.dt.int16
    assert in_ap.dtype == out_ap.dtype
    assert ap_utils.ap_is_contiguous(out_ap.ap[1:])
    assert ap_utils.ap_is_contiguous(idxs_ap.ap[1:])
    assert in_ap.ap[-1][1] == out_ap.ap[-1][1] == elem_size
    assert out_ap.ap[0][1] * out_ap.ap[1][1] == round_up_to_multiple(num_idxs, 128)
    assert in_ap.ap[0][0] == elem_step
    stride_bytes = elem_step * mybir.dt.size(in_ap.dtype)
    stride_bytes_256 = exact_div(stride_bytes, 256)
    assert stride_bytes_256 < 256

    _in_ap = eng.lower_ap_dma(in_ap, for_custom_bir_dma=True)
    _idxs_ap = eng.lower_ap(idxs_ap)
    _out_ap = eng.lower_ap(out_ap)
    inst = eng.add_instruction(
        mybir.InstDMAGatherAnt(
            name=eng.bass.get_next_instruction_name(),
            ins=[*_in_ap, _idxs_ap, eng.lower_val_access(eng.to_reg(num_idxs_reg))],
            outs=[_out_ap],
            transpose=False,
            num_idxs=num_idxs,
            elem_size=elem_size,
            stride_bytes_256=stride_bytes_256,
            gen_mode=int(prepare_only),
            single_packet=True,
            queue_num=queue_num,
            sbuf_tokens_per_rank=0,
            sbuf_free_dim_per_rank=0,
            sbuf_free_dim_pad_per_rank=0,
            sbuf_byte_offset=0,
        )
    )
    if prepare_only:
        assert sem is not None
        inst.then_inc(sem, 16)
        return eng._track_prepare_only(inst, queue_num)
    return inst


def build_program(nch, nchunk):
    import concourse.bacc as bacc
    import concourse.mybir as mybir
    from concourse.tile import TileContext

    fp32 = mybir.dt.float32
    bf16 = mybir.dt.bfloat16
    i16 = mybir.dt.int16

    nch = np.asarray(nch, np.int64).reshape(NWIN, 2)
    colmap, segments, nchunk2, meta = _schedule(nch)
    assert nchunk2 == nchunk

    nc = bacc.Bacc(num_swdge_queues=NQ)

    ht_d = nc.declare_dram_parameter("ht", [DIN, LPAD], bf16, isOutput=False)
    wp_d = nc.declare_dram_parameter("wp", [DIN, DOUT], bf16, isOutput=False)
    bp_d = nc.declare_dram_parameter("bp", [128, DOUT], fp32, isOutput=False)
    idx_d = nc.declare_dram_parameter("idx", [128, nchunk * 8], i16, isOutput=False)
    sv_d = nc.declare_dram_parameter("svals", [128, nchunk * 128], bf16, isOutput=False)
    out_d = nc.declare_dram_parameter("out", [LPAD, DOUT], fp32, isOutput=True)

    xshard = nc.dram_tensor("xshard", [LPAD, XW], bf16)
    xfullA = nc.dram_tensor("xfullA", [TA, XW], bf16, addr_space="Shared")
    xfullB = nc.dram_tensor("xfullB", [TB, XW], bf16, addr_space="Shared")
    ss_dram = nc.dram_tensor("ss_dram", [LPAD], fp32)

    NCHK49 = LPAD // 128   # 49 phase-1 row chunks
    CHKA = SPLITA // 128   # 25

    maxseg = max(cnt for _, cnt, _ in segments)

    with TileContext(nc) as tc:
        with (
            tc.tile_pool(name="big", bufs=1) as big,
            tc.tile_pool(name="consts", bufs=1) as consts,
            tc.tile_pool(name="gin", bufs=8) as gin,
            tc.tile_pool(name="spool", bufs=3) as spool,
            tc.tile_pool(name="epi", bufs=3) as epi,
        ):
            # ---------- phase 1: X shard ----------
            ones = consts.tile([128, 1], bf16)
            nc.vector.memset(ones[:], 1.0)
            wp_t = consts.tile([128, DOUT], bf16)
            nc.sync.dma_start(out=wp_t[:], in_=wp_d[:])
            bp_t = consts.tile([128, DOUT], fp32)
            nc.sync.dma_start(out=bp_t[:], in_=bp_d[:])

            # edge-stream preload (overlaps phase 1)
            ix_t = consts.tile([128, nchunk * 8], i16)
            nc.scalar.dma_start(out=ix_t[:], in_=idx_d[:])

            ht_t = big.tile([128, LPAD], bf16)
            nc.sync.dma_start(out=ht_t[:], in_=ht_d[:])
            hsq_t = big.tile([128, LPAD], bf16)
            nc.vector.tensor_tensor(out=hsq_t[:], in0=ht_t[:], in1=ht_t[:],
                                    op=mybir.AluOpType.mult)

            with tc.tile_pool(name="p1psum", bufs=2, space="PSUM") as p1ps:
                ss_sb = consts.tile([1, LPAD], fp32)
                for c0 in range(0, LPAD, 512):
                    w = min(512, LPAD - c0)
                    ssp = p1ps.tile([1, 512], fp32, space="PSUM", tag="ssp")
                    nc.tensor.matmul(out=ssp[:, :w], lhsT=ones[:],
                                     rhs=hsq_t[:, c0:c0 + w],
                                     start=True, stop=True)
                    nc.vector.tensor_copy(out=ss_sb[:, c0:c0 + w], in_=ssp[:, :w])

                # [1, 6272] -> [128, 49] across partitions via a DRAM bounce
                nc.sync.dma_start(out=ss_dram[:], in_=ss_sb[:])
                rn_in = consts.tile([128, NCHK49], fp32)
                nc.sync.dma_start(
                    out=rn_in[:],
                    in_=ss_dram[:].rearrange("(c p) -> p c", p=128),
                )
                sq_t = consts.tile([128, NCHK49], fp32)
                nc.scalar.activation(out=sq_t[:], in_=rn_in[:],
                                     func=mybir.ActivationFunctionType.Sqrt)
                rn_t = consts.tile([128, NCHK49], fp32)
                nc.vector.reciprocal(out=rn_t[:], in_=sq_t[:])

                xsbA = big.tile([128, CHKA * XW], bf16)
                nc.vector.memset(xsbA[:], 0.0)
                xsbB = big.tile([128, (NCHK49 - CHKA) * XW], bf16)
                nc.vector.memset(xsbB[:], 0.0)
                for c in range(NCHK49):
                    xp = p1ps.tile([128, DOUT], fp32, space="PSUM", tag="xp")
                    nc.tensor.matmul(out=xp[:], lhsT=ht_t[:, c * 128:(c + 1) * 128],
                                     rhs=wp_t[:], start=True, stop=True)
                    if c < CHKA:
                        xtgt = xsbA[:, c * XW:c * XW + DOUT]
                    else:
                        xtgt = xsbB[:, (c - CHKA) * XW:(c - CHKA) * XW + DOUT]
                    nc.vector.scalar_tensor_tensor(
                        out=xtgt,
                        in0=xp[:], scalar=rn_t[:, c:c + 1], in1=bp_t[:],
                        op0=mybir.AluOpType.mult, op1=mybir.AluOpType.add)
                    if c == CHKA - 1:
                        nc.sync.dma_start(
                            out=xshard[:SPLITA].rearrange("(c p) f -> p c f", p=128),
                            in_=xsbA[:].rearrange("p (c f) -> p c f", f=XW),
                        )
                        nc.gpsimd.collective_compute(
                            "AllGather", mybir.AluOpType.bypass,
                            ins=[xshard[:SPLITA]], outs=[xfullA[:]],
                            replica_groups=[list(range(NCORE))],
                        )
                nc.sync.dma_start(
                    out=xshard[SPLITA:].rearrange("(c p) f -> p c f", p=128),
                    in_=xsbB[:].rearrange("p (c f) -> p c f", f=XW),
                )
                nc.gpsimd.collective_compute(
                    "AllGather", mybir.AluOpType.bypass,
                    ins=[xshard[SPLITA:]], outs=[xfullB[:]],
                    replica_groups=[list(range(NCORE))],
                )

            # ---------- phase 3: gather + PE segment-sum ----------
            with tc.tile_pool(name="wpsum", bufs=8, space="PSUM") as wps_pool:
                ps = {}      # win -> psum tile
                callno = 0
                for (c0, cnt, h) in segments:
                    table = xfullA[:, :DOUT] if h == 0 else xfullB[:, :DOUT]
                    s_t = spool.tile([128, maxseg * 128], bf16, tag="s")
                    nc.sync.dma_start(out=s_t[:, :cnt * 128],
                                      in_=sv_d[:, c0 * 128:(c0 + cnt) * 128])
                    done = 0
                    while done < cnt:
                        n = min(CALLCAP, cnt - done)
                        g_t = gin.tile([128, CALLCAP * DOUT], bf16, tag="g")
                        _dma_gather_128(
                            nc.gpsimd,
                            out_ap=g_t[:, :n * DOUT].rearrange(
                                "p (c f) -> p c f", f=DOUT),
                            in_ap=table,
                            idxs_ap=ix_t[:, (c0 + done) * 8:(c0 + done + n) * 8],
                            num_idxs=n * 128, num_idxs_reg=n * 128,
                            elem_size=DOUT, elem_step=XW,
                            queue_num=callno % NQ)
                        callno += 1
                        for j in range(n):
                            col = c0 + done + j
                            w, is_start, is_stop = meta[col]
                            if is_start:
                                ps[w] = wps_pool.tile([128, 512], fp32,
                                                      space="PSUM", tag="ps",
                                                      name=f"ps{w}")
                            nc.tensor.matmul(
                                out=ps[w][:, :DOUT],
                                lhsT=s_t[:, (col - c0) * 128:(col - c0 + 1) * 128],
                                rhs=g_t[:, j * DOUT:(j + 1) * DOUT],
                                start=is_start, stop=is_stop)
                            if is_stop:
                                o_t = epi.tile([128, DOUT], fp32, tag="o")
                                nc.scalar.activation(
                                    out=o_t[:], in_=ps[w][:, :DOUT],
                                    func=mybir.ActivationFunctionType.Lrelu,
                                    alpha=SLOPE)
                                nc.sync.dma_start(
                                    out=out_d[w * WIN:(w + 1) * WIN, :],
                                    in_=o_t[:])
                                del ps[w]
                        done += n

    nc.compile()
    return nc


# ---------------- runner ----------------
_CACHE = {}


def _get_runner(meta):
    key = (meta["nch"], meta["nchunk"])
    if key in _CACHE:
        return _CACHE[key]

    import jax
    import concourse.mybir as mybir
    from concourse import bass2jax
    from concourse.bass2jax import _bass_exec_p, partition_id_tensor
    from jax.experimental.shard_map import shard_map
    from jax.sharding import Mesh, NamedSharding, PartitionSpec

    nc = build_program(*key)
    bass2jax.install_neuronx_cc_hook()

    partition_name = nc.partition_id_tensor.name if nc.partition_id_tensor else None
    in_names, out_names, out_avals = [], [], []
    for alloc in nc.m.functions[0].allocations:
        if not isinstance(alloc, mybir.MemoryLocationSet):
            continue
        name = alloc.memorylocations[0].name
        if alloc.kind == "ExternalInput":
            if name != partition_name:
                in_names.append(name)
        elif alloc.kind == "ExternalOutput":
            out_names.append(name)
            out_avals.append(jax.core.ShapedArray(tuple(alloc.tensor_shape),
                                                  mybir.dt.np(alloc.dtype)))
    n_params = len(in_names)
    all_in = in_names + out_names
    if partition_name is not None:
        all_in.append(partition_name)

    def _body(*args):
        operands = list(args)
        if partition_name is not None:
            operands.append(partition_id_tensor())
        outs = _bass_exec_p.bind(
            *operands, out_avals=tuple(out_avals), in_names=tuple(all_in),
            out_names=tuple(out_names), lowering_input_output_aliases=(),
            sim_require_finite=False, sim_require_nnan=False, nc=nc)
        return tuple(outs)

    devices = jax.devices()[:NCORE]
    mesh = Mesh(np.asarray(devices), ("core",))
    nin = n_params + len(out_names)
    fn = jax.jit(
        shard_map(_body, mesh=mesh, in_specs=(PartitionSpec("core"),) * nin,
                  out_specs=(PartitionSpec("core"),) * len(out_names),
                  check_rep=False),
        keep_unused=True)
    sharding = NamedSharding(mesh, PartitionSpec("core"))

    runner = dict(nc=nc, fn=fn, in_names=in_names, out_names=out_names,
                  out_avals=out_avals, sharding=sharding, mesh=mesh)
    _CACHE[key] = runner
    return runner


def run_on_hw(in_maps, meta, device_args=None):
    """Execute on the 8 cores; returns (out_full [50000,64] f32, runner, device_args)."""
    import jax
    r = _get_runner(meta)
    if device_args is None:
        device_args = prepare_device_args(r, in_maps)
    outs = r["fn"](*device_args)
    jax.block_until_ready(outs)
    out = np.asarray(outs[r["out_names"].index("out")])  # [8*LPAD, 64]
    out = out.reshape(NCORE, LPAD, DOUT)[:, :RPC, :].reshape(N, DOUT)
    return out, r, device_args


def prepare_device_args(r, in_maps):
    import jax
    args = []
    for name in r["in_names"]:
        cat = np.concatenate([np.asarray(m[name]) for m in in_maps], axis=0)
        args.append(jax.device_put(cat, r["sharding"]))
    for aval in r["out_avals"]:
        z = np.zeros((NCORE * aval.shape[0], *aval.shape[1:]), aval.dtype)
        args.append(jax.device_put(z, r["sharding"]))
    return args


def kernel(H, rows, cols, vals, gamma, beta, run_mean, run_var, W, b):
    in_maps, meta = host_prep(H, rows, cols, vals, gamma, beta, run_mean, run_var, W, b)
    out, _, _ = run_on_hw(in_maps, meta)
    return out


# revision 10
# speedup vs baseline: 1.1239x; 1.1239x over previous
"""GCN layer (normalize -> BN(eval) -> Linear -> SpMM -> LeakyReLU) on 8 TRN2 cores.

Self-contained: host-side preprocessing (sharding / edge sorting / BN folding),
Bass/Tile program builder, and SPMD runner.

Strategy (per core, SPMD):
  - nodes sharded 8 x 6250 (padded to 6272 = 49*128 local rows)
  - phase 1: X_shard = rn * (H_shard @ W') + b'   (bf16, PE matmul; row-norm via
    ones-matmul + Rsqrt; BN folded into W'/b' on host)
  - X AllGathered in TWO slices (locals [0,3200) -> table A [25600,128],
    [3200,6272) -> table B [24576,128]); the slice split doubles as the int16
    gather-index split and lets AG1 overlap the tail of phase 1.
  - phase 3: edges sorted by (dest window of 128 rows, table half); chunks of
    128 edges; chunks per (win, half) = ceil(max-over-cores/128). Windows are
    grouped in quads; per (quad, half) the chunks form one contiguous idx
    segment gathered with <=1024-idx dma_gather calls round-robined over 4
    SWDGE queues (~3ns/desc vs ~9 single-queue; fewer calls cut the ~2.5us
    Pool-engine issue cost per call). Host-prebuilt scatter matrices
    S[128e,128d] (val at dloc, bf16) stream from DRAM per segment; one PE
    matmul per chunk accumulates into the window's full-bank [128,512] PSUM
    tile (bank per window => interleaved window lifetimes stay sound).
    Epilogue = one scalar-engine Lrelu activation PSUM->SBUF, then DMA out.
"""

import os
import sys

import numpy as np

for _p in ("/opt/trn_rl_repo", "/root/.axon_site/_ro/trn_rl_repo"):
    if _p not in sys.path and os.path.isdir(_p):
        sys.path.insert(0, _p)

import ml_dtypes  # noqa: E402

BF16 = ml_dtypes.bfloat16

# ---------------- problem constants (hardcoded per contract) ----------------
N = 50000
E = 800000
DIN = 128
DOUT = 64
BN_EPS = 1e-5
SLOPE = 0.01

NCORE = 8
RPC = N // NCORE          # 6250 real rows per core
LPAD = 6272               # 49*128 padded local rows
WIN = 128                 # dest-window rows (S matrix width / psum partitions)
NWIN = LPAD // WIN        # 49 windows/core
SPLITA = 3200             # local rows [0, 3200) -> table A (25 phase-1 chunks)
SPLITB = LPAD - SPLITA    # 3072 -> table B (24 chunks)
TA = NCORE * SPLITA       # 25600 rows (< 32768: int16-addressable)
TB = NCORE * SPLITB       # 24576 rows
XW = 128                  # X rows padded to 128 bf16 cols (256B stride)
NQ = 4                    # SWDGE queues (max 4)
CALLCAP = 8               # gather chunks per dma_gather call (1024 idx cap)
QUADW = 4                 # windows per gather-segment group


def _quads():
    qs = []
    w = 0
    while w < NWIN:
        qs.append(list(range(w, min(w + QUADW, NWIN))))
        w += QUADW
    return qs


def _schedule(nch):
    """nch [NWIN, 2] -> (colmap [NWIN,2], segments, nchunk, chunk_meta).

    Chunk columns ordered (quad, half, window, k). segments = per (quad, half):
    (col0, count, half). chunk_meta[col] = (win, start, stop)."""
    colmap = np.zeros((NWIN, 2), np.int64)
    segments = []
    col = 0
    for quad in _quads():
        for h in (0, 1):
            c0 = col
            for w in quad:
                colmap[w, h] = col
                col += int(nch[w, h])
            if col > c0:
                segments.append((c0, col - c0, h))
    nchunk = col
    meta = [None] * nchunk
    for w in range(NWIN):
        tot = int(nch[w, 0] + nch[w, 1])
        if tot == 0:
            continue
        ci = 0
        for h in (0, 1):
            for k in range(int(nch[w, h])):
                c = int(colmap[w, h]) + k
                meta[c] = (w, ci == 0, ci == tot - 1)
                ci += 1
    return colmap, segments, nchunk, meta


# ---------------- host preprocessing ----------------
def host_prep(H, rows, cols, vals, gamma, beta, run_mean, run_var, W, b):
    """Build the 8 per-core input maps + the static chunk schedule."""
    H = np.asarray(H, np.float32)
    rows = np.asarray(rows, np.int64)
    cols = np.asarray(cols, np.int64)
    vals = np.asarray(vals, np.float32)

    # BN fold: X = Hn @ W' + b'  with W' = diag(scale) W, b' = b + (beta-mean*scale)W
    scale = np.asarray(gamma, np.float32) / np.sqrt(np.asarray(run_var, np.float32) + BN_EPS)
    Wp = (np.asarray(W, np.float32) * scale[:, None]).astype(BF16)          # [128, 64]
    bp = (np.asarray(b, np.float32)
          + (np.asarray(beta, np.float32) - np.asarray(run_mean, np.float32) * scale)
          @ np.asarray(W, np.float32)).astype(np.float32)                    # [64]
    bp_tile = np.tile(bp[None, :], (128, 1)).astype(np.float32)              # [128, 64]

    core = rows // RPC
    lr = rows - core * RPC              # local dest row, 0..6249
    win = lr // WIN                     # 0..48
    dloc = lr - win * WIN               # 0..127
    m_src = cols // RPC
    lsrc = cols - m_src * RPC           # local source row, 0..6249
    half = (lsrc >= SPLITA).astype(np.int64)
    loc = np.where(half == 0, m_src * SPLITA + lsrc,
                   m_src * SPLITB + (lsrc - SPLITA))

    # ---- per-(core, win, half) slot assignment ----
    key = (core * NWIN + win) * 2 + half
    order = np.argsort(key, kind="stable")
    counts = np.bincount(key, minlength=NCORE * NWIN * 2)
    starts = np.zeros(NCORE * NWIN * 2 + 1, np.int64)
    np.cumsum(counts, out=starts[1:])
    j_in_run = np.arange(E, dtype=np.int64) - starts[key[order]]

    c3 = counts.reshape(NCORE, NWIN, 2)
    m8 = c3.max(axis=0)                           # [NWIN, 2]
    nch = ((m8 + 127) // 128).astype(np.int64)    # chunks per (win, half)
    colmap, segments, nchunk, _meta = _schedule(nch)

    e_core = core[order]
    e_win = win[order]
    e_half = half[order]
    e_dloc = dloc[order]
    e_loc = loc[order]
    e_val = vals[order]
    e_k = j_in_run // 128
    lane = (j_in_run % 128).astype(np.int64)
    col = colmap[e_win, e_half] + e_k

    idx16 = np.zeros((NCORE, 128, nchunk), np.int16)
    idx16[e_core, lane, col] = e_loc.astype(np.int16)
    # pre-built scatter matrices: svals[core, lane, col*128 + dloc] = val
    svals = np.zeros((NCORE, 128, nchunk * 128), BF16)
    svals[e_core, lane, col * 128 + e_dloc] = e_val.astype(BF16)

    # wrapped-16 idx, built per chunk (chunk c occupies cols [8c, 8c+8)):
    # idx_w[core, 16g + l%16, 8c + l//16] = idx16[core, l, c]
    tmp = idx16.reshape(NCORE, 8, 16, nchunk).transpose(0, 2, 3, 1)  # [NC,16,c,8]
    tmp = tmp.reshape(NCORE, 16, nchunk * 8)
    idx_w = np.ascontiguousarray(np.tile(tmp, (1, 8, 1)))

    # ---- H transposed shards, bf16 ----
    in_maps = []
    for m in range(NCORE):
        ht = np.zeros((DIN, LPAD), BF16)
        ht[:, :RPC] = H[m * RPC:(m + 1) * RPC].T.astype(BF16)
        ht[0, RPC:] = BF16(1.0)  # pad rows get norm 1 -> finite rn, X never read
        in_maps.append(dict(
            ht=ht,
            wp=Wp,
            bp=bp_tile,
            idx=np.ascontiguousarray(idx_w[m]),
            svals=np.ascontiguousarray(svals[m]),
        ))
    meta = dict(nch=tuple(int(x) for x in nch.reshape(-1)), nchunk=nchunk)
    return in_maps, meta


# ---------------- bass program ----------------
def _dma_gather_128(eng, out_ap, in_ap, idxs_ap, num_idxs, num_idxs_reg,
                    elem_size, elem_step, queue_num=0, prepare_only=False,
                    sem=None):
    """bass's dma_gather minus the 256B elem assert (transpose-only per the
    ucode; non-transpose packets are byte-granular, only the table row STRIDE
    must be a multiple of 256B). DRAM-source, non-transpose only."""
    import concourse.mybir as mybir
    from concourse.bass import round_up_to_multiple, exact_div
    from concourse import ap_utils

    assert idxs_ap.dtype == mybir.dt.int16
    assert in_ap.dtype == out_ap.dtype
    assert ap_utils.ap_is_contiguous(out_ap.ap[1:])
    assert ap_utils.ap_is_contiguous(idxs_ap.ap[1:])
    assert in_ap.ap[-1][1] == out_ap.ap[-1][1] == elem_size
    assert out_ap.ap[0][1] * out_ap.ap[1][1] == round_up_to_multiple(num_idxs, 128)
    assert in_ap.ap[0][0] == elem_step
    stride_bytes = elem_step * mybir.dt.size(in_ap.dtype)
    stride_bytes_256 = exact_div(stride_bytes, 256)
    assert stride_bytes_256 < 256

    _in_ap = eng.lower_ap_dma(in_ap, for_custom_bir_dma=True)
    _idxs_ap = eng.lower_ap(idxs_ap)
    _out_ap = eng.lower_ap(out_ap)
    inst = eng.add_instruction(
        mybir.InstDMAGatherAnt(
            name=eng.bass.get_next_instruction_name(),
            ins=[*_in_ap, _idxs_ap, eng.lower_val_access(eng.to_reg(num_idxs_reg))],
            outs=[_out_ap],
            transpose=False,
            num_idxs=num_idxs,
            elem_size=elem_size,
            stride_bytes_256=stride_bytes_256,
            gen_mode=int(prepare_only),
            single_packet=True,
            queue_num=queue_num,
            sbuf_tokens_per_rank=0,
            sbuf_free_dim_per_rank=0,
            sbuf_free_dim_pad_per_rank=0,
            sbuf_byte_offset=0,
        )
    )
    if prepare_only:
        assert sem is not None
        inst.then_inc(sem, 16)
        return eng._track_prepare_only(inst, queue_num)
    return inst


def build_program(nch, nchunk):
    import concourse.bacc as bacc
    import concourse.mybir as mybir
    from concourse.tile import TileContext

    fp32 = mybir.dt.float32
    bf16 = mybir.dt.bfloat16
    i16 = mybir.dt.int16

    nch = np.asarray(nch, np.int64).reshape(NWIN, 2)
    colmap, segments, nchunk2, meta = _schedule(nch)
    assert nchunk2 == nchunk

    nc = bacc.Bacc(num_swdge_queues=NQ)

    ht_d = nc.declare_dram_parameter("ht", [DIN, LPAD], bf16, isOutput=False)
    wp_d = nc.declare_dram_parameter("wp", [DIN, DOUT], bf16, isOutput=False)
    bp_d = nc.declare_dram_parameter("bp", [128, DOUT], fp32, isOutput=False)
    idx_d = nc.declare_dram_parameter("idx", [128, nchunk * 8], i16, isOutput=False)
    sv_d = nc.declare_dram_parameter("svals", [128, nchunk * 128], bf16, isOutput=False)
    out_d = nc.declare_dram_parameter("out", [LPAD, DOUT], fp32, isOutput=True)

    xshard = nc.dram_tensor("xshard", [LPAD, XW], bf16)
    xfullA = nc.dram_tensor("xfullA", [TA, XW], bf16, addr_space="Shared")
    xfullB = nc.dram_tensor("xfullB", [TB, XW], bf16, addr_space="Shared")
    ss_dram = nc.dram_tensor("ss_dram", [LPAD], fp32)

    NCHK49 = LPAD // 128   # 49 phase-1 row chunks
    CHKA = SPLITA // 128   # 25

    maxseg = max(cnt for _, cnt, _ in segments)

    with TileContext(nc) as tc:
        with (
            tc.tile_pool(name="big", bufs=1) as big,
            tc.tile_pool(name="consts", bufs=1) as consts,
            tc.tile_pool(name="gin", bufs=8) as gin,
            tc.tile_pool(name="spool", bufs=3) as spool,
            tc.tile_pool(name="epi", bufs=3) as epi,
        ):
            # ---------- phase 1: X shard ----------
            ones = consts.tile([128, 1], bf16)
            nc.vector.memset(ones[:], 1.0)
            wp_t = consts.tile([128, DOUT], bf16)
            nc.sync.dma_start(out=wp_t[:], in_=wp_d[:])
            bp_t = consts.tile([128, DOUT], fp32)
            nc.sync.dma_start(out=bp_t[:], in_=bp_d[:])

            # edge-stream preload (overlaps phase 1)
            ix_t = consts.tile([128, nchunk * 8], i16)
            nc.scalar.dma_start(out=ix_t[:], in_=idx_d[:])

            ht_t = big.tile([128, LPAD], bf16)
            nc.sync.dma_start(out=ht_t[:], in_=ht_d[:])
            hsq_t = big.tile([128, LPAD], bf16)
            nc.vector.tensor_tensor(out=hsq_t[:], in0=ht_t[:], in1=ht_t[:],
                                    op=mybir.AluOpType.mult)

            with tc.tile_pool(name="p1psum", bufs=2, space="PSUM") as p1ps:
                ss_sb = consts.tile([1, LPAD], fp32)
                for c0 in range(0, LPAD, 512):
                    w = min(512, LPAD - c0)
                    ssp = p1ps.tile([1, 512], fp32, space="PSUM", tag="ssp")
                    nc.tensor.matmul(out=ssp[:, :w], lhsT=ones[:],
                                     rhs=hsq_t[:, c0:c0 + w],
                                     start=True, stop=True)
                    nc.vector.tensor_copy(out=ss_sb[:, c0:c0 + w], in_=ssp[:, :w])

                # [1, 6272] -> [128, 49] across partitions via a DRAM bounce
                nc.sync.dma_start(out=ss_dram[:], in_=ss_sb[:])
                rn_in = consts.tile([128, NCHK49], fp32)
                nc.sync.dma_start(
                    out=rn_in[:],
                    in_=ss_dram[:].rearrange("(c p) -> p c", p=128),
                )
                sq_t = consts.tile([128, NCHK49], fp32)
                nc.scalar.activation(out=sq_t[:], in_=rn_in[:],
                                     func=mybir.ActivationFunctionType.Sqrt)
                rn_t = consts.tile([128, NCHK49], fp32)
                nc.vector.reciprocal(out=rn_t[:], in_=sq_t[:])

                xsbA = big.tile([128, CHKA * XW], bf16)
                nc.vector.memset(xsbA[:], 0.0)
                xsbB = big.tile([128, (NCHK49 - CHKA) * XW], bf16)
                nc.vector.memset(xsbB[:], 0.0)
                for c in range(NCHK49):
                    xp = p1ps.tile([128, DOUT], fp32, space="PSUM", tag="xp")
                    nc.tensor.matmul(out=xp[:], lhsT=ht_t[:, c * 128:(c + 1) * 128],
                                     rhs=wp_t[:], start=True, stop=True)
                    if c < CHKA:
                        xtgt = xsbA[:, c * XW:c * XW + DOUT]
                    else:
                        xtgt = xsbB[:, (c - CHKA) * XW:(c - CHKA) * XW + DOUT]
                    nc.vector.scalar_tensor_tensor(
                        out=xtgt,
                        in0=xp[:], scalar=rn_t[:, c:c + 1], in1=bp_t[:],
                        op0=mybir.AluOpType.mult, op1=mybir.AluOpType.add)
                    if c == CHKA - 1:
                        nc.sync.dma_start(
                            out=xshard[:SPLITA].rearrange("(c p) f -> p c f", p=128),
                            in_=xsbA[:].rearrange("p (c f) -> p c f", f=XW),
                        )
                        nc.gpsimd.collective_compute(
                            "AllGather", mybir.AluOpType.bypass,
                            ins=[xshard[:SPLITA]], outs=[xfullA[:]],
                            replica_groups=[list(range(NCORE))],
                        )
                nc.sync.dma_start(
                    out=xshard[SPLITA:].rearrange("(c p) f -> p c f", p=128),
                    in_=xsbB[:].rearrange("p (c f) -> p c f", f=XW),
                )
                nc.gpsimd.collective_compute(
                    "AllGather", mybir.AluOpType.bypass,
                    ins=[xshard[SPLITA:]], outs=[xfullB[:]],
                    replica_groups=[list(range(NCORE))],
                )

            # ---------- phase 3: gather + PE segment-sum ----------
            with tc.tile_pool(name="wpsum", bufs=8, space="PSUM") as wps_pool:
                ps = {}      # win -> psum tile
                callno = 0
                for (c0, cnt, h) in segments:
                    table = xfullA[:, :DOUT] if h == 0 else xfullB[:, :DOUT]
                    s_t = spool.tile([128, maxseg * 128], bf16, tag="s")
                    nc.sync.dma_start(out=s_t[:, :cnt * 128],
                                      in_=sv_d[:, c0 * 128:(c0 + cnt) * 128])
                    done = 0
                    while done < cnt:
                        n = min(CALLCAP, cnt - done)
                        g_t = gin.tile([128, CALLCAP * DOUT], bf16, tag="g")
                        _dma_gather_128(
                            nc.gpsimd,
                            out_ap=g_t[:, :n * DOUT].rearrange(
                                "p (c f) -> p c f", f=DOUT),
                            in_ap=table,
                            idxs_ap=ix_t[:, (c0 + done) * 8:(c0 + done + n) * 8],
                            num_idxs=n * 128, num_idxs_reg=n * 128,
                            elem_size=DOUT, elem_step=XW,
                            queue_num=callno % NQ)
                        callno += 1
                        for j in range(n):
                            col = c0 + done + j
                            w, is_start, is_stop = meta[col]
                            if is_start:
                                ps[w] = wps_pool.tile([128, 512], fp32,
                                                      space="PSUM", tag="ps",
                                                      name=f"ps{w}")
                            nc.tensor.matmul(
                                out=ps[w][:, :DOUT],
                                lhsT=s_t[:, (col - c0) * 128:(col - c0 + 1) * 128],
                                rhs=g_t[:, j * DOUT:(j + 1) * DOUT],
                                start=is_start, stop=is_stop)
                            if is_stop:
                                o_t = epi.tile([128, DOUT], fp32, tag="o")
                                nc.scalar.activation(
                                    out=o_t[:], in_=ps[w][:, :DOUT],
                                    func=mybir.ActivationFunctionType.Lrelu,
                                    alpha=SLOPE)
                                nc.sync.dma_start(
                                    out=out_d[w * WIN:(w + 1) * WIN, :],
                                    in_=o_t[:])
                                del ps[w]
                        done += n

    nc.compile()
    return nc


# ---------------- runner ----------------
_CACHE = {}


def _get_runner(meta):
    key = (meta["nch"], meta["nchunk"])
    if key in _CACHE:
        return _CACHE[key]

    import jax
    import concourse.mybir as mybir
    from concourse import bass2jax
    from concourse.bass2jax import _bass_exec_p, partition_id_tensor
    from jax.experimental.shard_map import shard_map
    from jax.sharding import Mesh, NamedSharding, PartitionSpec

    nc = build_program(*key)
    bass2jax.install_neuronx_cc_hook()

    partition_name = nc.partition_id_tensor.name if nc.partition_id_tensor else None
    in_names, out_names, out_avals = [], [], []
    for alloc in nc.m.functions[0].allocations:
        if not isinstance(alloc, mybir.MemoryLocationSet):
            continue
        name = alloc.memorylocations[0].name
        if alloc.kind == "ExternalInput":
            if name != partition_name:
                in_names.append(name)
        elif alloc.kind == "ExternalOutput":
            out_names.append(name)
            out_avals.append(jax.core.ShapedArray(tuple(alloc.tensor_shape),
                                                  mybir.dt.np(alloc.dtype)))
    n_params = len(in_names)
    all_in = in_names + out_names
    if partition_name is not None:
        all_in.append(partition_name)

    def _body(*args):
        operands = list(args)
        if partition_name is not None:
            operands.append(partition_id_tensor())
        outs = _bass_exec_p.bind(
            *operands, out_avals=tuple(out_avals), in_names=tuple(all_in),
            out_names=tuple(out_names), lowering_input_output_aliases=(),
            sim_require_finite=False, sim_require_nnan=False, nc=nc)
        return tuple(outs)

    devices = jax.devices()[:NCORE]
    mesh = Mesh(np.asarray(devices), ("core",))
    nin = n_params + len(out_names)
    fn = jax.jit(
        shard_map(_body, mesh=mesh, in_specs=(PartitionSpec("core"),) * nin,
                  out_specs=(PartitionSpec("core"),) * len(out_names),
                  check_rep=False),
        keep_unused=True)
    sharding = NamedSharding(mesh, PartitionSpec("core"))

    runner = dict(nc=nc, fn=fn, in_names=in_names, out_names=out_names,
                  out_avals=out_avals, sharding=sharding, mesh=mesh)
    _CACHE[key] = runner
    return runner


def run_on_hw(in_maps, meta, device_args=None):
    """Execute on the 8 cores; returns (out_full [50000,64] f32, runner, device_args)."""
    import jax
    r = _get_runner(meta)
    if device_args is None:
        device_args = prepare_device_args(r, in_maps)
    outs = r["fn"](*device_args)
    jax.block_until_ready(outs)
    out = np.asarray(outs[r["out_names"].index("out")])  # [8*LPAD, 64]
    out = out.reshape(NCORE, LPAD, DOUT)[:, :RPC, :].reshape(N, DOUT)
    return out, r, device_args


def prepare_device_args(r, in_maps):
    import jax
    args = []
    for name in r["in_names"]:
        cat = np.concatenate([np.asarray(m[name]) for m in in_maps], axis=0)
        args.append(jax.device_put(cat, r["sharding"]))
    for aval in r["out_avals"]:
        z = np.zeros((NCORE * aval.shape[0], *aval.shape[1:]), aval.dtype)
        args.append(jax.device_put(z, r["sharding"]))
    return args


def kernel(H, rows, cols, vals, gamma, beta, run_mean, run_var, W, b):
    in_maps, meta = host_prep(H, rows, cols, vals, gamma, beta, run_mean, run_var, W, b)
    out, _, _ = run_on_hw(in_maps, meta)
    return out
